# revision 21
# baseline (speedup 1.0000x reference)
"""Causal single-head attention on 8 trn2 NeuronCores.

Problem: x[4,2048,1024], Wq/Wk/Wv[1024,64] ->
  softmax(causal((x@Wq)@(x@Wk).T / 32)) @ (x@Wv)  -> [4,2048,64]

Sharding: 8 cores = 4 batches x 2 query-shards. Zigzag query split for
causal load balance: shard A handles query blocks {0,3} (of 512 rows),
shard B handles {1,2}. Each core redundantly computes K/V for the key
blocks it needs from a host-transposed x[b].T.

SPMD uniformity: one program for all 8 cores. Per-core differences are
absorbed into data:
  - xt column-block permutation (A: [0,1,3,2], B: [1,0,2,3]) puts each
    core's diagonal (q==k) blocks at fixed program slots: q blocks live
    at xt slots 0 and 2, so pair (qslot I, kslot 0) and (II, kslot 2)
    are always the diagonal pairs; their triangular strip masks are a
    compile-time triu pattern.
  - a tiny per-core bias input kills fully-masked (dummy) pairs via the
    exp() bias: exp(s - 1e5) == 0.

On-chip layout (scores kept transposed so softmax denominators and the
attention*V product are plain matmuls):
  qT[64,1024], kvT[128,2048] = (Wq|Wk|Wv)^T @ xt   (f32r matmuls)
  scoresT[k,q] = kT_tile.T @ qT                     (per 128k x 512q tile)
  exp via ScalarE with per-pair bias; diag strips masked by triu mult
  out_augT[65,512q] += v_aug_tile.T @ expT  where v_aug = [v | 1] gives
    the softmax denominator for free in row 64
  finalize: PE-transpose out_augT, divide rows by denominator, DMA out.
"""

import os
import sys

import numpy as np

if "/opt/trn_rl_repo" not in sys.path and os.path.isdir("/opt/trn_rl_repo"):
    sys.path.insert(0, "/opt/trn_rl_repo")

import concourse.bacc as bacc
import concourse.mybir as mybir
import concourse.tile as tile
from concourse.bass_utils import run_bass_kernel_spmd

B, S, E, H = 4, 2048, 1024, 64
BLK = 512  # kv/q block (4 blocks per sequence)
NCORES = 8
NE = E // 128  # 8 e-tiles
F32 = mybir.dt.float32
F32R = mybir.dt.float32r
FEXP = mybir.ActivationFunctionType.Exp

# per-shard: query blocks and xt column-block permutation
QBLOCKS = {0: (0, 3), 1: (1, 2)}
PERM = {0: (0, 1, 3, 2), 1: (1, 0, 2, 3)}
# program-fixed pair list: (qslot, kslot); pairs 0 and 4 are diagonal
PAIRS = ((0, 0), (0, 1), (1, 0), (1, 1), (1, 2), (1, 3))
DIAG = (0, 4)
NEG = -1.0e5


def _build():
    nc = bacc.Bacc("TRN2", target_bir_lowering=False, debug=False, num_devices=NCORES)

    xt = nc.dram_tensor("xt", [E, S], F32, kind="ExternalInput").ap()
    # weights host-prearranged to SBUF layout: [p, (e h)], wkv then wq
    wqkv = nc.dram_tensor("wqkv", [128, NE * 3 * H], F32, kind="ExternalInput").ap()
    bias2 = nc.dram_tensor("bias2", [128, 8], F32, kind="ExternalInput").ap()
    triu = nc.dram_tensor("triu", [128, 128], F32, kind="ExternalInput").ap()
    ones2 = nc.dram_tensor("ones2", [128, 32], F32, kind="ExternalInput").ap()
    idmat = nc.dram_tensor("idmat", [128, 128], F32, kind="ExternalInput").ap()
    out = nc.dram_tensor("out", [2 * BLK, H], F32, kind="ExternalOutput").ap()

    with tile.TileContext(nc) as tc:
        with (
            tc.tile_pool(name="const", bufs=1) as cpool,
            tc.tile_pool(name="xt", bufs=4) as xtpool,
            tc.tile_pool(name="exp", bufs=4) as expool,
            tc.tile_pool(name="fin", bufs=2) as finpool,
            tc.tile_pool(name="kvps", bufs=1, space="PSUM") as kvps_pool,
            tc.tile_pool(name="qps", bufs=1, space="PSUM") as qps_pool,
            tc.tile_pool(name="vtps", bufs=1, space="PSUM") as vtps_pool,
            tc.tile_pool(name="stps", bufs=3, space="PSUM") as stps_pool,
            tc.tile_pool(name="avps", bufs=2, space="PSUM") as avps_pool,
        ):
            # ---- constants ----
            wqkv_sb = cpool.tile([128, NE * 3 * H], F32R)
            nc.sync.dma_start(out=wqkv_sb, in_=wqkv.bitcast(F32R))
            wkv_sb = wqkv_sb[:, 0 : NE * 2 * H]
            wq_sb = wqkv_sb[:, NE * 2 * H :]
            bias2_sb = cpool.tile([128, 8], F32)
            nc.gpsimd.dma_start(out=bias2_sb, in_=bias2)
            triu_sb = cpool.tile([128, 128], F32)
            nc.gpsimd.dma_start(out=triu_sb, in_=triu)
            idmat_sb = cpool.tile([128, 128], F32R)
            nc.gpsimd.dma_start(out=idmat_sb, in_=idmat.bitcast(F32R))

            # persistent buffers
            kvT_sb = cpool.tile([128, S], F32R)  # rows 0:64 kT, 64:128 vT
            qT_sb = cpool.tile([64, 2 * BLK], F32R)
            # [v(64) | 1 | 1] per k-tile; 66-wide so the fp32r stationary
            # free count is even (col 65 is a harmless denominator dup)
            VA = H + 2
            vaug_sb = cpool.tile([128, 16 * VA], F32R)
            ones_sb = cpool.tile([128, 32], F32R)
            nc.gpsimd.dma_start(out=ones_sb, in_=ones2.bitcast(F32R))
            nc.gpsimd.tensor_copy(
                vaug_sb.rearrange("p (t c) -> p t c", t=16)[:, :, H : H + 2],
                ones_sb.rearrange("p (t c) -> p t c", c=2),
            )

            av_ps = [None, None]  # group accumulators, created lazily

            def proj_slot(s):
                """project xt column-block s -> kvT_sb[:, s*BLK:], qT if q slot,
                and v-natural tiles into vaug."""
                xt_sb = xtpool.tile([128, NE * BLK], F32R, name=f"xt_{s}", tag="xt")
                xt3 = xt.bitcast(F32R).rearrange("(e p) s -> p e s", p=128)
                for h in range(4):
                    nc.sync.dma_start(
                        out=xt_sb.rearrange("p (e c) -> p e c", e=NE)[
                            :, h * 2 : (h + 1) * 2, :
                        ],
                        in_=xt3[:, h * 2 : (h + 1) * 2, s * BLK : (s + 1) * BLK],
                    )
                xts = [xt_sb[:, e * BLK : (e + 1) * BLK] for e in range(NE)]
                kv_ps = kvps_pool.tile([128, BLK], F32, name=f"kvps_{s}", tag="kv")
                q_ps = None
                if s in (0, 2):
                    q_ps = qps_pool.tile([64, BLK], F32, name=f"qps_{s}", tag="q")
                for e in range(NE):
                    nc.tensor.matmul(
                        kv_ps,
                        wkv_sb[:, e * 128 : (e + 1) * 128],
                        xts[e],
                        start=(e == 0),
                        stop=(e == NE - 1),
                    )
                    if q_ps is not None:
                        nc.tensor.matmul(
                            q_ps,
                            wq_sb[:, e * H : (e + 1) * H],
                            xts[e],
                            start=(e == 0),
                            stop=(e == NE - 1),
                        )
                nc.vector.tensor_copy(kvT_sb[:, s * BLK : (s + 1) * BLK], kv_ps)
                if q_ps is not None:
                    qs = s // 2
                    nc.vector.tensor_copy(qT_sb[:, qs * BLK : (qs + 1) * BLK], q_ps)
                # v natural tiles for the AV product
                for j in range(4):
                    t = s * 4 + j
                    vt_ps = vtps_pool.tile([128, H + 2], F32R, name=f"vt_{t}", tag="vt")
                    nc.tensor.transpose(
                        vt_ps[:, 0:H],
                        kvT_sb[64:128, t * 128 : (t + 1) * 128],
                        idmat_sb[64:128, 64:128],
                    )
                    nc.vector.tensor_copy(
                        vaug_sb[:, t * VA : t * VA + H], vt_ps[:, 0:H]
                    )

            def do_pair(p):
                qslot, kslot = PAIRS[p]
                diag = p in DIAG
                first = p in (0, 2)
                last = p in (1, 5)
                for j in range(4):
                    st_ps = stps_pool.tile([128, BLK], F32, name=f"st_{p}_{j}", tag="st")
                    nc.tensor.matmul(
                        st_ps,
                        kvT_sb[0:64, kslot * BLK + j * 128 : kslot * BLK + (j + 1) * 128],
                        qT_sb[0:64, qslot * BLK : (qslot + 1) * BLK],
                        start=True,
                        stop=True,
                    )
                    ex = expool.tile([128, BLK], F32R, name=f"ex_{p}_{j}", tag="ex")
                    if diag and j > 0:
                        nc.scalar.activation(
                            ex[:, 0 : j * 128], st_ps[:, 0 : j * 128], FEXP, bias=bias2_sb[:, 6:7]
                        )
                        nc.scalar.activation(
                            ex[:, j * 128 :], st_ps[:, j * 128 :], FEXP, bias=0.0
                        )
                    else:
                        nc.scalar.activation(
                            ex, st_ps, FEXP, bias=bias2_sb[:, p : p + 1]
                        )
                    if diag:
                        nc.gpsimd.tensor_tensor(
                            ex[:, j * 128 : (j + 1) * 128],
                            ex[:, j * 128 : (j + 1) * 128],
                            triu_sb,
                            mybir.AluOpType.mult,
                        )
                    if av_ps[qslot] is None:
                        av_ps[qslot] = avps_pool.tile(
                            [H + 2, BLK], F32, name=f"av_{qslot}", tag="av"
                        )
                    t = kslot * 4 + j
                    nc.tensor.matmul(
                        av_ps[qslot],
                        vaug_sb[:, t * VA : (t + 1) * VA],
                        ex,
                        start=(first and j == 0),
                        stop=(last and j == 3),
                        skip_group_check=True,
                    )

            def finalize(qs):
                oT_sb = finpool.tile([H + 2, BLK], F32R, name=f"oT_{qs}", tag="oT")
                nc.vector.tensor_copy(oT_sb, av_ps[qs])
                for t in range(4):
                    tr_ps = vtps_pool.tile([128, H + 2], F32R, name=f"tr_{qs}_{t}", tag="vt")
                    nc.tensor.transpose(
                        tr_ps,
                        oT_sb[:, t * 128 : (t + 1) * 128],
                        idmat_sb[0 : H + 2, 0 : H + 2],
                    )
                    rden = finpool.tile([128, 1], F32, name=f"rd_{qs}_{t}", tag="rd")
                    nc.vector.reciprocal(rden, tr_ps[:, H : H + 1].bitcast(F32))
                    o_sb = finpool.tile([128, H], F32, name=f"o_{qs}_{t}", tag="o")
                    nc.vector.tensor_scalar_mul(o_sb, tr_ps[:, 0:H].bitcast(F32), rden)
                    r0 = qs * BLK + t * 128
                    nc.sync.dma_start(out=out[r0 : r0 + 128, :], in_=o_sb)

            proj_slot(0)
            do_pair(0)
            proj_slot(1)
            do_pair(1)
            finalize(0)
            proj_slot(2)
            do_pair(2)
            do_pair(3)
            do_pair(4)
            proj_slot(3)
            do_pair(5)
            finalize(1)

    nc.compile()
    return nc


_NC_CACHE = None
RUN_KWARGS = {}  # test harness may set {"trace": True}
LAST_RESULTS = None  # BassKernelResults of the most recent run


def kernel(x, Wq, Wk, Wv):
    global _NC_CACHE, LAST_RESULTS
    x = np.asarray(x, dtype=np.float32)
    Wq = np.asarray(Wq, dtype=np.float32)
    Wk = np.asarray(Wk, dtype=np.float32)
    Wv = np.asarray(Wv, dtype=np.float32)

    def to_sb(w):  # [E, h] -> [128, NE*h] with e-tiles side by side
        h = w.shape[1]
        return np.ascontiguousarray(
            w.reshape(NE, 128, h).transpose(1, 0, 2).reshape(128, NE * h)
        )

    wq_s = to_sb(Wq / np.float32(E**0.5))
    wkv = to_sb(np.concatenate([Wk, Wv], axis=1))
    wqkv = np.ascontiguousarray(np.concatenate([wkv, wq_s], axis=1))
    triu = np.triu(np.ones((128, 128), dtype=np.float32))
    ones2 = np.ones((128, 32), dtype=np.float32)
    idmat = np.eye(128, dtype=np.float32)

    in_maps = []
    for core in range(NCORES):
        b, shard = divmod(core, 2)
        perm = PERM[shard]
        xtf = x[b].T  # [E, S]
        xt = np.ascontiguousarray(
            np.concatenate([xtf[:, p * BLK : (p + 1) * BLK] for p in perm], axis=1)
        )
        qb = QBLOCKS[shard]
        bias2 = np.zeros((128, 8), dtype=np.float32)
        bias2[:, 6] = NEG
        for p, (qslot, kslot) in enumerate(PAIRS):
            if perm[kslot] > qb[qslot]:  # key block entirely in the future
                bias2[:, p] = NEG
        in_maps.append(
            dict(xt=xt, wqkv=wqkv, bias2=bias2, triu=triu, idmat=idmat, ones2=ones2)
        )

    if _NC_CACHE is None:
        _NC_CACHE = _build()
    res = run_bass_kernel_spmd(
        _NC_CACHE, in_maps, core_ids=list(range(NCORES)), **RUN_KWARGS
    )
    LAST_RESULTS = res

    out = np.empty((B, S, H), dtype=np.float32)
    for core in range(NCORES):
        b, shard = divmod(core, 2)
        o = res.results[core]["out"]
        for qs, blk in enumerate(QBLOCKS[shard]):
            out[b, blk * BLK : (blk + 1) * BLK, :] = o[qs * BLK : (qs + 1) * BLK, :]
    return out
